# revision 14
# baseline (speedup 1.0000x reference)
"""TT interpolation kernel, same-cell-pairing variant.

Like kernel.py (host-built stacked bf16 table + dma_gather + bf16 DVE
combine), but points sharing a G-table cell are PAIRED host-side so one
gather descriptor serves two points: per 16 output points, the G side
gathers 11 entries (5 pair slots + 6 single slots) instead of 16, cutting
total descriptors per 32 points from 64 to 54 (-15.6% on the exclusive
DMA-engines device, which is the bottleneck).

Layout per (partition, 16-jout unit): jout rows 0..9 = 5 pairs (2 points
per gathered slot, distinct weight quads), rows 10..15 = singles. The H
side keeps one slot per point, ordered by jout. Host assigns points to
(partition, unit, row) and un-permutes y at the end.
"""

import numpy as np
import ml_dtypes

import concourse.bacc as bacc
import concourse.mybir as mybir
import concourse.tile as tile
from concourse import library_config
from concourse.bass_utils import run_bass_kernel_spmd

F32 = mybir.dt.float32
BF16 = mybir.dt.bfloat16
I16 = mybir.dt.int16
OP = mybir.AluOpType

NCORES = 8
B = 262144
BS = B // NCORES          # 32768 points per core
P = 128
J = BS // P               # 256 point-columns (jout) per partition
NU = J // 16              # 16 jout-units per partition
N = 128
R = 16
TE = 2 * N * N
ES = 128                  # bf16 elems per entry (64 payload + 64 pad)
# G-side grouping zones per 16-jout unit: (group_size, group_count), in row
# order. sum(size*count) must be 16. Groups of size s share one gathered
# entry among s points (same G cell). Chosen adaptively from the input.
ZONE_CONFIGS = [
    [(3, 2), (2, 3), (1, 4)],   # 9 G-slots/unit (needs 4096 triples, 6144 pairs)
    [(2, 5), (1, 6)],           # 11 G-slots/unit (needs 10240 pairs)
    [(2, 4), (1, 8)],           # 12
    [(1, 16)],                  # 16 (unpaired fallback)
]
ZONES = ZONE_CONFIGS[0]
GS = sum(cnt for _, cnt in ZONES)
# chunk sizes in 16-jout units (small ends for start/tail latency)
CHUNK_U = [1, 1, 2, 2, 2, 2, 2, 2, 1, 1]
assert sum(CHUNK_U) == NU
NCH = len(CHUNK_U)
LGC = GS * P // 16        # G-list cols per unit
LHC = 16 * P // 16        # 128 H-list cols per unit
LROWS = 32


def _set_zones(zones):
    global ZONES, GS, LGC
    ZONES = zones
    assert sum(g * c for g, c in zones) == 16
    GS = sum(cnt for _, cnt in zones)
    LGC = GS * P // 16


_CACHED = None
_CACHED_ZONES = None


def _build_nc():
    nc = bacc.Bacc("TRN2")

    tbl = nc.dram_tensor("tbl", [TE, ES], BF16, kind="ExternalInput")
    lstg = nc.dram_tensor("lstg", [LROWS, NU * LGC], I16, kind="ExternalInput")
    lsth = nc.dram_tensor("lsth", [LROWS, NU * LHC], I16, kind="ExternalInput")
    w4g = nc.dram_tensor("w4g", [P, NU * 16 * 4], BF16, kind="ExternalInput")
    w4h = nc.dram_tensor("w4h", [P, NU * 16 * 4], BF16, kind="ExternalInput")
    y_pm = nc.dram_tensor("y_pm", [P, J], F32, kind="ExternalOutput")

    with tile.TileContext(nc) as tc:
        with (
            tc.tile_pool(name="per", bufs=1) as pe,
            tc.tile_pool(name="gbuf", bufs=3) as gb,
            tc.tile_pool(name="cbuf", bufs=2) as cb,
        ):
            nc.gpsimd.load_library(library_config.mlp)

            LG = pe.tile([LROWS, NU * LGC], I16)
            LH = pe.tile([LROWS, NU * LHC], I16)
            WG = pe.tile([P, NU, 16, 4], BF16)
            WH = pe.tile([P, NU, 16, 4], BF16)
            ysb = pe.tile([P, J], F32)
            # first chunk's G-list loads first so gather 0 starts early
            nc.sync.dma_start(LG[:, 0:LGC], lstg[:, 0:LGC])
            nc.sync.dma_start(LG[:, LGC:], lstg[:, LGC:])
            nc.sync.dma_start(LH[:], lsth[:])

            u0 = 0
            for ch, cu in enumerate(CHUNK_U):
                ngi = cu * GS * P          # G gather slots this chunk
                nhi = cu * 16 * P
                gG = gb.tile([P, cu * GS, ES], BF16, tag="gG",
                             padded_shape=[P, 2 * GS, ES])
                nc.gpsimd.dma_gather(
                    gG[:], tbl[:], LG[:, u0 * LGC : u0 * LGC + cu * LGC],
                    ngi, ngi, ES, queue_num=0, single_packet=False,
                )
                gH = gb.tile([P, cu * 16, ES], BF16, tag="gH",
                             padded_shape=[P, 2 * 16, ES])
                nc.gpsimd.dma_gather(
                    gH[:], tbl[:], LH[:, u0 * LHC : u0 * LHC + cu * LHC],
                    nhi, nhi, ES, queue_num=0, single_packet=False,
                )
                if ch == 0:
                    nc.sync.dma_start(
                        WG[:].rearrange("p u r c -> p (u r c)"), w4g[:]
                    )
                    nc.sync.dma_start(
                        WH[:].rearrange("p u r c -> p (u r c)"), w4h[:]
                    )

                # ---- G side ----
                # DVE APs allow at most 3 free dims after adjacent-stride
                # merging; the pair views' unit stride (GS slots) cannot merge
                # with the slot dim, so loop over the chunk's units (<= 2).
                gGv = gG[:].rearrange("p (u s) e -> p u s e", s=GS)
                uG = cb.tile([P, cu, 16, R], BF16, tag="uG",
                             padded_shape=[P, 2, 16, R])
                for u in range(cu):
                    sbase = 0
                    rbase = 0
                    for zi, (gsz, cnt) in enumerate(ZONES):
                        # cnt groups of gsz points sharing one gathered slot:
                        # jout rows rbase..rbase+gsz*cnt (i-major, sub-minor)
                        gp = (
                            gGv[:, u, sbase : sbase + cnt, 0:64]
                            .unsqueeze(2)
                            .broadcast_to([P, cnt, gsz, 64])
                            .rearrange("p i s (k c) -> p i s k c", c=4)
                        )
                        wp = (
                            WG[:, u0 + u, rbase : rbase + gsz * cnt]
                            .rearrange("p (i s) c -> p i s c", s=gsz)
                            .unsqueeze(3)
                            .broadcast_to([P, cnt, gsz, R, 4])
                        )
                        mp = cb.tile([P, cnt, gsz, R, 4], BF16,
                                     tag=f"mp{u}z{zi}")
                        nc.vector.tensor_tensor(mp[:], gp, wp, OP.mult)
                        m2p = cb.tile([P, cnt, gsz, R, 2], BF16,
                                      tag=f"m2p{u}z{zi}")
                        nc.vector.tensor_tensor(
                            m2p[:], mp[:, :, :, :, 0:2], mp[:, :, :, :, 2:4],
                            OP.add,
                        )
                        nc.vector.tensor_tensor(
                            uG[:, u, rbase : rbase + gsz * cnt].rearrange(
                                "p (i s) k -> p i s k", s=gsz
                            ),
                            m2p[:, :, :, :, 0],
                            m2p[:, :, :, :, 1],
                            OP.add,
                        )
                        sbase += cnt
                        rbase += gsz * cnt
                # ---- H side (one slot per jout) ----
                gh = (
                    gH[:]
                    .rearrange("p (u r) e -> p u r e", r=16)[:, :, :, 0:64]
                    .rearrange("p u r (k c) -> p u r k c", c=4)
                )
                wh = (
                    WH[:, u0 : u0 + cu]
                    .unsqueeze(3)
                    .broadcast_to([P, cu, 16, R, 4])
                )
                mh = cb.tile([P, cu, 16, R, 4], BF16, tag="mh",
                             padded_shape=[P, 2, 16, R, 4])
                nc.vector.tensor_tensor(mh[:], gh, wh, OP.mult)
                m2h = cb.tile([P, cu, 16, R, 2], BF16, tag="m2h",
                              padded_shape=[P, 2, 16, R, 2])
                nc.vector.tensor_tensor(
                    m2h[:], mh[:, :, :, :, 0:2], mh[:, :, :, :, 2:4], OP.add
                )
                uH = cb.tile([P, cu, 16, R], BF16, tag="uH",
                             padded_shape=[P, 2, 16, R])
                nc.vector.tensor_tensor(
                    uH[:], m2h[:, :, :, :, 0], m2h[:, :, :, :, 1], OP.add
                )
                # ---- dot ----
                pr = cb.tile([P, cu, 16, R], BF16, tag="pr",
                             padded_shape=[P, 2, 16, R])
                nc.vector.tensor_tensor(pr[:], uG[:], uH[:], OP.mult)
                nc.vector.tensor_reduce(
                    ysb[:, 16 * u0 : 16 * (u0 + cu)].rearrange(
                        "p (u r) -> p u r", r=16
                    ),
                    pr[:],
                    mybir.AxisListType.X,
                    OP.add,
                )
                u0 += cu
                if ch == NCH - 2:
                    nc.sync.dma_start(
                        y_pm[:, 0 : 16 * u0], ysb[:, 0 : 16 * u0]
                    )

            nc.sync.dma_start(y_pm[:, 16 * (NU - CHUNK_U[-1]) :],
                              ysb[:, 16 * (NU - CHUNK_U[-1]) :])

    nc.finalize()
    return nc


def _build_tables(core0, core1, core2, core3):
    G = np.einsum("nc,cmk->nmk", core0[0], core1)
    H = np.einsum("cn,kmc->nmk", core3[:, :, 0], core2)
    out = np.zeros((2, N * N, ES), dtype=ml_dtypes.bfloat16)
    for t, A in enumerate((G, H)):
        Ap = np.pad(A, ((0, 1), (0, 1), (0, 0)), mode="edge")
        corn = np.stack(
            [
                Ap[0:N, 0:N],
                Ap[0:N, 1 : N + 1],
                Ap[1 : N + 1, 0:N],
                Ap[1 : N + 1, 1 : N + 1],
            ],
            axis=-1,
        )
        out[t, :, 0:64] = corn.reshape(N * N, 64).astype(ml_dtypes.bfloat16)
    return np.ascontiguousarray(out.reshape(TE, ES))


def _wrap_list(flat):
    """slot i -> [i%16, i//16], replicated to LROWS rows."""
    lw = flat.reshape(-1, 16).T
    return np.ascontiguousarray(np.tile(lw, (LROWS // 16, 1)))


def _group_by_cell(idxG):
    """Greedy same-cell grouping into the ZONES quota (groups of 3/2/1).
    Returns {gsz: array [n_groups, gsz]} or None if supply is short."""
    sizes = sorted({g for g, _ in ZONES}, reverse=True)
    need = {g: c * P * NU for g, c in ZONES}
    order = np.argsort(idxG, kind="stable")
    sidx = idxG[order]
    bnd = np.flatnonzero(np.r_[True, sidx[1:] != sidx[:-1]])
    counts = np.diff(np.r_[bnd, len(sidx)])
    pools = {g: [] for g in sizes}
    for s, c in zip(bnd, counts):
        g = order[s : s + c]
        o = 0
        for gsz in sizes:
            if gsz == 1:
                break
            while c - o >= gsz:
                pools[gsz].append(g[o : o + gsz])
                o += gsz
        if c - o:
            pools[1].extend(g[i : i + 1] for i in range(o, c))
    out = {}
    for gsz in sizes:
        if gsz == 1:
            continue
        grp = (
            np.stack(pools[gsz])
            if pools[gsz]
            else np.empty((0, gsz), dtype=np.int64)
        )
        n = need.get(gsz, 0)
        if len(grp) < n:
            return None
        # demote surplus groups: keep a prefix for the next-smaller pool,
        # remainder becomes singles
        smaller = [g for g in sizes if 1 < g < gsz]
        nxt = smaller[0] if smaller else 1
        for surplus in grp[n:]:
            if nxt > 1:
                pools[nxt].append(surplus[:nxt])
            for i in range(nxt if nxt > 1 else 0, gsz):
                pools[1].append(surplus[i : i + 1])
        out[gsz] = grp[:n]
    singles = np.concatenate(pools[1]) if pools[1] else np.empty(0, np.int64)
    if len(singles) != need.get(1, 0):
        return None
    out[1] = singles.reshape(-1, 1)
    return out


def _prep_core(idxG, idxH, wG, wH):
    """Group points on the G cell per ZONES and assign to
    (partition, unit, row). Returns lstg, lsth, w4g, w4h, perm where
    perm[p, jout] = original point index at that output position."""
    groups = _group_by_cell(idxG)
    assert groups is not None

    # assignment: unit-major, then partition; per unit the jout rows are
    # zone-ordered (each zone's groups i-major, sub-minor)
    perm_parts = []        # [p, u, rows] pieces per zone
    gcell_parts = []       # [u, slots, p] pieces per zone
    for gsz, cnt in ZONES:
        grp = groups[gsz].reshape(NU, P, cnt, gsz)   # [u, p, i, s]
        perm_parts.append(
            grp.transpose(1, 0, 2, 3).reshape(P, NU, gsz * cnt)
        )
        gcell_parts.append(
            idxG[grp[:, :, :, 0]].transpose(0, 2, 1)  # [u, i, p]
        )
    perm = np.concatenate(perm_parts, axis=2).reshape(P, J)
    gcell = np.concatenate(gcell_parts, axis=1).astype(np.int16)  # [u, GS, p]
    lstg = _wrap_list(gcell.reshape(-1))

    # H slot list: slot i = (u * 16 + r) * P + p, cell of point perm[p, 16u+r]
    hp = idxH[perm]                                 # [p, jout]
    hcell = hp.reshape(P, NU, 16).transpose(1, 2, 0)  # [u, r, p]
    lsth = _wrap_list(np.ascontiguousarray(hcell).reshape(-1))

    # weights: w4g[p, u, r, c] = G-quad of the point at (p, u, r)
    w4g = wG[perm].reshape(P, NU, 16, 4).astype(ml_dtypes.bfloat16)
    w4h = wH[perm].reshape(P, NU, 16, 4).astype(ml_dtypes.bfloat16)
    return (
        lstg,
        lsth,
        np.ascontiguousarray(w4g.reshape(P, NU * 16 * 4)),
        np.ascontiguousarray(w4h.reshape(P, NU * 16 * 4)),
        perm,
    )


_PERMS = None


def _prep_inputs(x, core0, core1, core2, core3):
    global _PERMS
    core0 = np.asarray(core0, dtype=np.float32)
    core1 = np.asarray(core1, dtype=np.float32)
    core2 = np.asarray(core2, dtype=np.float32)
    core3 = np.asarray(core3, dtype=np.float32)
    tbl = _build_tables(core0, core1, core2, core3)

    x = np.asarray(x, dtype=np.float32)
    xc = np.clip(
        (x + np.float32(1.0)) * np.float32(0.5) * np.float32(N - 1),
        np.float32(0.0),
        np.float32(N - 1),
    )
    lo = np.minimum(np.floor(xc), np.float32(N - 2)).astype(np.int32)
    fr = xc - lo.astype(np.float32)
    a = np.float32(1.0) - fr

    idxG = (lo[:, 0] * N + lo[:, 1]).astype(np.int32)
    idxH = (N * N + lo[:, 3] * N + lo[:, 2]).astype(np.int32)
    wG = np.stack(
        [a[:, 0] * a[:, 1], a[:, 0] * fr[:, 1], fr[:, 0] * a[:, 1], fr[:, 0] * fr[:, 1]],
        axis=-1,
    )
    wH = np.stack(
        [a[:, 3] * a[:, 2], a[:, 3] * fr[:, 2], fr[:, 3] * a[:, 2], fr[:, 3] * fr[:, 2]],
        axis=-1,
    )

    # choose the most aggressive zone config the input distribution supports
    for cfg in ZONE_CONFIGS:
        _set_zones(cfg)
        if all(
            _group_by_cell(idxG[c * BS : (c + 1) * BS].astype(np.int16))
            is not None
            for c in range(NCORES)
        ):
            break

    in_maps = []
    _PERMS = []
    for cix in range(NCORES):
        s = slice(cix * BS, (cix + 1) * BS)
        lstg, lsth, w4gc, w4hc, perm = _prep_core(
            idxG[s].astype(np.int16), idxH[s].astype(np.int16), wG[s], wH[s]
        )
        _PERMS.append(perm)
        in_maps.append(
            {"tbl": tbl, "lstg": lstg, "lsth": lsth, "w4g": w4gc, "w4h": w4hc}
        )
    return in_maps


def kernel(x, core0, core1, core2, core3):
    global _CACHED, _CACHED_ZONES
    in_maps = _prep_inputs(x, core0, core1, core2, core3)  # sets ZONES
    if _CACHED is None or _CACHED_ZONES != ZONES:
        _CACHED = _build_nc()
        _CACHED_ZONES = ZONES
    nc = _CACHED
    res = run_bass_kernel_spmd(nc, in_maps, core_ids=list(range(NCORES)))
    out = np.empty(B, dtype=np.float32)
    for cix in range(NCORES):
        y_pm = np.asarray(res.results[cix]["y_pm"])   # [p, jout]
        dst = out[cix * BS : (cix + 1) * BS]
        dst[_PERMS[cix].reshape(-1)] = y_pm.reshape(-1)
    return out


# revision 15
# speedup vs baseline: 1.0359x; 1.0359x over previous
"""TT interpolation kernel with same-cell gather grouping.

y[b] = sum_k u_k * v_k with u = bilinear interp of the joint table
G(x0,x1), v of H(x3,x2); both tables are host-built from the TT cores and
shipped stacked as 256B bf16 entries [16 k x 4 corners]. The device does
one dma_gather stream plus a bf16 DVE combine (weight multiply with the
corner axis packed innermost for the 2x mode, corner add tree, u*v,
k-reduce).

The gather descriptor count is the bottleneck (exclusive DMA device,
22.76 ns per 256B descriptor / 16 engines), so points sharing a G-table
cell are GROUPED host-side: one gathered entry serves all group members,
each with its own bilinear weight quad. Per 16 output points the G side
gathers 9 slots (2 triples + 3 pairs + 4 singles, zone config chosen
adaptively from the input distribution) instead of 16; the H side keeps
one slot per point ordered by output position so every device-side read
stays affine. Host assigns points to (partition, unit, row) slots and
un-permutes y afterward. Descriptors per 32 points: 64 -> 50 (-22%).
"""

import numpy as np
import ml_dtypes

import concourse.bacc as bacc
import concourse.mybir as mybir
import concourse.tile as tile
from concourse import library_config
from concourse.bass_utils import run_bass_kernel_spmd

F32 = mybir.dt.float32
BF16 = mybir.dt.bfloat16
I16 = mybir.dt.int16
OP = mybir.AluOpType

NCORES = 8
B = 262144
BS = B // NCORES          # 32768 points per core
P = 128
J = BS // P               # 256 point-columns (jout) per partition
NU = J // 16              # 16 jout-units per partition
N = 128
R = 16
TE = 2 * N * N
ES = 128                  # bf16 elems per entry (64 payload + 64 pad)
# G-side grouping zones per 16-jout unit: (group_size, group_count), in row
# order. sum(size*count) must be 16. Groups of size s share one gathered
# entry among s points (same G cell). Chosen adaptively from the input.
ZONE_CONFIGS = [
    [(3, 2), (2, 3), (1, 4)],   # 9 G-slots/unit (needs 4096 triples, 6144 pairs)
    [(2, 5), (1, 6)],           # 11 G-slots/unit (needs 10240 pairs)
    [(2, 4), (1, 8)],           # 12
    [(1, 16)],                  # 16 (unpaired fallback)
]
ZONES = ZONE_CONFIGS[0]
GS = sum(cnt for _, cnt in ZONES)
# chunk sizes in 16-jout units (small ends for start/tail latency)
CHUNK_U = [1, 1, 2, 2, 2, 2, 2, 2, 1, 1]
assert sum(CHUNK_U) == NU
NCH = len(CHUNK_U)
LGC = GS * P // 16        # G-list cols per unit
LHC = 16 * P // 16        # 128 H-list cols per unit
LROWS = 32


def _set_zones(zones):
    global ZONES, GS, LGC
    ZONES = zones
    assert sum(g * c for g, c in zones) == 16
    GS = sum(cnt for _, cnt in zones)
    LGC = GS * P // 16


_CACHED = None
_CACHED_ZONES = None


def _build_nc():
    nc = bacc.Bacc("TRN2")

    tbl = nc.dram_tensor("tbl", [TE, ES], BF16, kind="ExternalInput")
    lstg = nc.dram_tensor("lstg", [LROWS, NU * LGC], I16, kind="ExternalInput")
    lsth = nc.dram_tensor("lsth", [LROWS, NU * LHC], I16, kind="ExternalInput")
    w4g = nc.dram_tensor("w4g", [P, NU * 16 * 4], BF16, kind="ExternalInput")
    w4h = nc.dram_tensor("w4h", [P, NU * 16 * 4], BF16, kind="ExternalInput")
    y_pm = nc.dram_tensor("y_pm", [P, J], F32, kind="ExternalOutput")

    with tile.TileContext(nc) as tc:
        with (
            tc.tile_pool(name="per", bufs=1) as pe,
            tc.tile_pool(name="gbuf", bufs=3) as gb,
            tc.tile_pool(name="cbuf", bufs=2) as cb,
        ):
            nc.gpsimd.load_library(library_config.mlp)

            LG = pe.tile([LROWS, NU * LGC], I16)
            LH = pe.tile([LROWS, NU * LHC], I16)
            WG = pe.tile([P, NU, 16, 4], BF16)
            WH = pe.tile([P, NU, 16, 4], BF16)
            ysb = pe.tile([P, J], F32)
            # first chunk's G-list loads first so gather 0 starts early
            nc.sync.dma_start(LG[:, 0:LGC], lstg[:, 0:LGC])
            nc.sync.dma_start(LG[:, LGC:], lstg[:, LGC:])
            nc.sync.dma_start(LH[:], lsth[:])

            u0 = 0
            for ch, cu in enumerate(CHUNK_U):
                ngi = cu * GS * P          # G gather slots this chunk
                nhi = cu * 16 * P
                gG = gb.tile([P, cu * GS, ES], BF16, tag="gG",
                             padded_shape=[P, 2 * GS, ES])
                nc.gpsimd.dma_gather(
                    gG[:], tbl[:], LG[:, u0 * LGC : u0 * LGC + cu * LGC],
                    ngi, ngi, ES, queue_num=0, single_packet=False,
                )
                gH = gb.tile([P, cu * 16, ES], BF16, tag="gH",
                             padded_shape=[P, 2 * 16, ES])
                nc.gpsimd.dma_gather(
                    gH[:], tbl[:], LH[:, u0 * LHC : u0 * LHC + cu * LHC],
                    nhi, nhi, ES, queue_num=0, single_packet=False,
                )
                if ch == 0:
                    nc.sync.dma_start(
                        WG[:].rearrange("p u r c -> p (u r c)"), w4g[:]
                    )
                    nc.sync.dma_start(
                        WH[:].rearrange("p u r c -> p (u r c)"), w4h[:]
                    )

                # ---- G side ----
                # DVE APs allow at most 3 free dims after adjacent-stride
                # merging; the pair views' unit stride (GS slots) cannot merge
                # with the slot dim, so loop over the chunk's units (<= 2).
                gGv = gG[:].rearrange("p (u s) e -> p u s e", s=GS)
                uG = cb.tile([P, cu, 16, R], BF16, tag="uG",
                             padded_shape=[P, 2, 16, R])
                for u in range(cu):
                    sbase = 0
                    rbase = 0
                    for zi, (gsz, cnt) in enumerate(ZONES):
                        # cnt groups of gsz points sharing one gathered slot:
                        # jout rows rbase..rbase+gsz*cnt (i-major, sub-minor)
                        gp = (
                            gGv[:, u, sbase : sbase + cnt, 0:64]
                            .unsqueeze(2)
                            .broadcast_to([P, cnt, gsz, 64])
                            .rearrange("p i s (k c) -> p i s k c", c=4)
                        )
                        wp = (
                            WG[:, u0 + u, rbase : rbase + gsz * cnt]
                            .rearrange("p (i s) c -> p i s c", s=gsz)
                            .unsqueeze(3)
                            .broadcast_to([P, cnt, gsz, R, 4])
                        )
                        mp = cb.tile([P, cnt, gsz, R, 4], BF16,
                                     tag=f"mp{u}z{zi}")
                        nc.vector.tensor_tensor(mp[:], gp, wp, OP.mult)
                        m2p = cb.tile([P, cnt, gsz, R, 2], BF16,
                                      tag=f"m2p{u}z{zi}")
                        nc.vector.tensor_tensor(
                            m2p[:], mp[:, :, :, :, 0:2], mp[:, :, :, :, 2:4],
                            OP.add,
                        )
                        nc.vector.tensor_tensor(
                            uG[:, u, rbase : rbase + gsz * cnt].rearrange(
                                "p (i s) k -> p i s k", s=gsz
                            ),
                            m2p[:, :, :, :, 0],
                            m2p[:, :, :, :, 1],
                            OP.add,
                        )
                        sbase += cnt
                        rbase += gsz * cnt
                # ---- H side (one slot per jout) ----
                gh = (
                    gH[:]
                    .rearrange("p (u r) e -> p u r e", r=16)[:, :, :, 0:64]
                    .rearrange("p u r (k c) -> p u r k c", c=4)
                )
                wh = (
                    WH[:, u0 : u0 + cu]
                    .unsqueeze(3)
                    .broadcast_to([P, cu, 16, R, 4])
                )
                mh = cb.tile([P, cu, 16, R, 4], BF16, tag="mh",
                             padded_shape=[P, 2, 16, R, 4])
                nc.vector.tensor_tensor(mh[:], gh, wh, OP.mult)
                m2h = cb.tile([P, cu, 16, R, 2], BF16, tag="m2h",
                              padded_shape=[P, 2, 16, R, 2])
                nc.vector.tensor_tensor(
                    m2h[:], mh[:, :, :, :, 0:2], mh[:, :, :, :, 2:4], OP.add
                )
                uH = cb.tile([P, cu, 16, R], BF16, tag="uH",
                             padded_shape=[P, 2, 16, R])
                nc.vector.tensor_tensor(
                    uH[:], m2h[:, :, :, :, 0], m2h[:, :, :, :, 1], OP.add
                )
                # ---- dot ----
                pr = cb.tile([P, cu, 16, R], BF16, tag="pr",
                             padded_shape=[P, 2, 16, R])
                nc.vector.tensor_tensor(pr[:], uG[:], uH[:], OP.mult)
                nc.vector.tensor_reduce(
                    ysb[:, 16 * u0 : 16 * (u0 + cu)].rearrange(
                        "p (u r) -> p u r", r=16
                    ),
                    pr[:],
                    mybir.AxisListType.X,
                    OP.add,
                )
                u0 += cu
                if ch == NCH - 2:
                    nc.sync.dma_start(
                        y_pm[:, 0 : 16 * u0], ysb[:, 0 : 16 * u0]
                    )

            nc.sync.dma_start(y_pm[:, 16 * (NU - CHUNK_U[-1]) :],
                              ysb[:, 16 * (NU - CHUNK_U[-1]) :])

    nc.finalize()
    return nc


def _build_tables(core0, core1, core2, core3):
    G = np.einsum("nc,cmk->nmk", core0[0], core1)
    H = np.einsum("cn,kmc->nmk", core3[:, :, 0], core2)
    out = np.zeros((2, N * N, ES), dtype=ml_dtypes.bfloat16)
    for t, A in enumerate((G, H)):
        Ap = np.pad(A, ((0, 1), (0, 1), (0, 0)), mode="edge")
        corn = np.stack(
            [
                Ap[0:N, 0:N],
                Ap[0:N, 1 : N + 1],
                Ap[1 : N + 1, 0:N],
                Ap[1 : N + 1, 1 : N + 1],
            ],
            axis=-1,
        )
        out[t, :, 0:64] = corn.reshape(N * N, 64).astype(ml_dtypes.bfloat16)
    return np.ascontiguousarray(out.reshape(TE, ES))


def _wrap_list(flat):
    """slot i -> [i%16, i//16], replicated to LROWS rows."""
    lw = flat.reshape(-1, 16).T
    return np.ascontiguousarray(np.tile(lw, (LROWS // 16, 1)))


def _group_by_cell(idxG):
    """Greedy same-cell grouping into the ZONES quota (groups of 3/2/1).
    Returns {gsz: array [n_groups, gsz]} or None if supply is short."""
    sizes = sorted({g for g, _ in ZONES}, reverse=True)
    need = {g: c * P * NU for g, c in ZONES}
    order = np.argsort(idxG, kind="stable")
    sidx = idxG[order]
    bnd = np.flatnonzero(np.r_[True, sidx[1:] != sidx[:-1]])
    counts = np.diff(np.r_[bnd, len(sidx)])
    pools = {g: [] for g in sizes}
    for s, c in zip(bnd, counts):
        g = order[s : s + c]
        o = 0
        for gsz in sizes:
            if gsz == 1:
                break
            while c - o >= gsz:
                pools[gsz].append(g[o : o + gsz])
                o += gsz
        if c - o:
            pools[1].extend(g[i : i + 1] for i in range(o, c))
    out = {}
    for gsz in sizes:
        if gsz == 1:
            continue
        grp = (
            np.stack(pools[gsz])
            if pools[gsz]
            else np.empty((0, gsz), dtype=np.int64)
        )
        n = need.get(gsz, 0)
        if len(grp) < n:
            return None
        # demote surplus groups: keep a prefix for the next-smaller pool,
        # remainder becomes singles
        smaller = [g for g in sizes if 1 < g < gsz]
        nxt = smaller[0] if smaller else 1
        for surplus in grp[n:]:
            if nxt > 1:
                pools[nxt].append(surplus[:nxt])
            for i in range(nxt if nxt > 1 else 0, gsz):
                pools[1].append(surplus[i : i + 1])
        out[gsz] = grp[:n]
    singles = np.concatenate(pools[1]) if pools[1] else np.empty(0, np.int64)
    if len(singles) != need.get(1, 0):
        return None
    out[1] = singles.reshape(-1, 1)
    return out


def _prep_core(idxG, idxH, wG, wH):
    """Group points on the G cell per ZONES and assign to
    (partition, unit, row). Returns lstg, lsth, w4g, w4h, perm where
    perm[p, jout] = original point index at that output position."""
    groups = _group_by_cell(idxG)
    assert groups is not None

    # assignment: unit-major, then partition; per unit the jout rows are
    # zone-ordered (each zone's groups i-major, sub-minor)
    perm_parts = []        # [p, u, rows] pieces per zone
    gcell_parts = []       # [u, slots, p] pieces per zone
    for gsz, cnt in ZONES:
        grp = groups[gsz].reshape(NU, P, cnt, gsz)   # [u, p, i, s]
        perm_parts.append(
            grp.transpose(1, 0, 2, 3).reshape(P, NU, gsz * cnt)
        )
        gcell_parts.append(
            idxG[grp[:, :, :, 0]].transpose(0, 2, 1)  # [u, i, p]
        )
    perm = np.concatenate(perm_parts, axis=2).reshape(P, J)
    gcell = np.concatenate(gcell_parts, axis=1).astype(np.int16)  # [u, GS, p]
    lstg = _wrap_list(gcell.reshape(-1))

    # H slot list: slot i = (u * 16 + r) * P + p, cell of point perm[p, 16u+r]
    hp = idxH[perm]                                 # [p, jout]
    hcell = hp.reshape(P, NU, 16).transpose(1, 2, 0)  # [u, r, p]
    lsth = _wrap_list(np.ascontiguousarray(hcell).reshape(-1))

    # weights: w4g[p, u, r, c] = G-quad of the point at (p, u, r)
    w4g = wG[perm].reshape(P, NU, 16, 4).astype(ml_dtypes.bfloat16)
    w4h = wH[perm].reshape(P, NU, 16, 4).astype(ml_dtypes.bfloat16)
    return (
        lstg,
        lsth,
        np.ascontiguousarray(w4g.reshape(P, NU * 16 * 4)),
        np.ascontiguousarray(w4h.reshape(P, NU * 16 * 4)),
        perm,
    )


_PERMS = None


def _prep_inputs(x, core0, core1, core2, core3):
    global _PERMS
    core0 = np.asarray(core0, dtype=np.float32)
    core1 = np.asarray(core1, dtype=np.float32)
    core2 = np.asarray(core2, dtype=np.float32)
    core3 = np.asarray(core3, dtype=np.float32)
    tbl = _build_tables(core0, core1, core2, core3)

    x = np.asarray(x, dtype=np.float32)
    xc = np.clip(
        (x + np.float32(1.0)) * np.float32(0.5) * np.float32(N - 1),
        np.float32(0.0),
        np.float32(N - 1),
    )
    lo = np.minimum(np.floor(xc), np.float32(N - 2)).astype(np.int32)
    fr = xc - lo.astype(np.float32)
    a = np.float32(1.0) - fr

    idxG = (lo[:, 0] * N + lo[:, 1]).astype(np.int32)
    idxH = (N * N + lo[:, 3] * N + lo[:, 2]).astype(np.int32)
    wG = np.stack(
        [a[:, 0] * a[:, 1], a[:, 0] * fr[:, 1], fr[:, 0] * a[:, 1], fr[:, 0] * fr[:, 1]],
        axis=-1,
    )
    wH = np.stack(
        [a[:, 3] * a[:, 2], a[:, 3] * fr[:, 2], fr[:, 3] * a[:, 2], fr[:, 3] * fr[:, 2]],
        axis=-1,
    )

    # choose the most aggressive zone config the input distribution supports
    for cfg in ZONE_CONFIGS:
        _set_zones(cfg)
        if all(
            _group_by_cell(idxG[c * BS : (c + 1) * BS].astype(np.int16))
            is not None
            for c in range(NCORES)
        ):
            break

    in_maps = []
    _PERMS = []
    for cix in range(NCORES):
        s = slice(cix * BS, (cix + 1) * BS)
        lstg, lsth, w4gc, w4hc, perm = _prep_core(
            idxG[s].astype(np.int16), idxH[s].astype(np.int16), wG[s], wH[s]
        )
        _PERMS.append(perm)
        in_maps.append(
            {"tbl": tbl, "lstg": lstg, "lsth": lsth, "w4g": w4gc, "w4h": w4hc}
        )
    return in_maps


def kernel(x, core0, core1, core2, core3):
    global _CACHED, _CACHED_ZONES
    in_maps = _prep_inputs(x, core0, core1, core2, core3)  # sets ZONES
    if _CACHED is None or _CACHED_ZONES != ZONES:
        _CACHED = _build_nc()
        _CACHED_ZONES = ZONES
    nc = _CACHED
    res = run_bass_kernel_spmd(nc, in_maps, core_ids=list(range(NCORES)))
    out = np.empty(B, dtype=np.float32)
    for cix in range(NCORES):
        y_pm = np.asarray(res.results[cix]["y_pm"])   # [p, jout]
        dst = out[cix * BS : (cix + 1) * BS]
        dst[_PERMS[cix].reshape(-1)] = y_pm.reshape(-1)
    return out


# revision 16
# speedup vs baseline: 1.0489x; 1.0126x over previous
"""TT interpolation kernel with same-cell gather grouping.

y[b] = sum_k u_k * v_k with u = bilinear interp of the joint table
G(x0,x1), v of H(x3,x2); both tables are host-built from the TT cores and
shipped stacked as 256B bf16 entries [16 k x 4 corners]. The device does
one dma_gather stream plus a bf16 DVE combine (weight multiply with the
corner axis packed innermost for the 2x mode, corner add tree, u*v,
k-reduce).

The gather descriptor count is the bottleneck (exclusive DMA device,
22.76 ns per 256B descriptor / 16 engines), so points sharing a G-table
cell are GROUPED host-side: one gathered entry serves all group members,
each with its own bilinear weight quad. Per 16 output points the G side
gathers 9 slots (2 triples + 3 pairs + 4 singles, zone config chosen
adaptively from the input distribution) instead of 16; the H side keeps
one slot per point ordered by output position so every device-side read
stays affine. Host assigns points to (partition, unit, row) slots and
un-permutes y afterward. Descriptors per 32 points: 64 -> 50 (-22%).
"""

import numpy as np
import ml_dtypes

import concourse.bacc as bacc
import concourse.mybir as mybir
import concourse.tile as tile
from concourse import library_config
from concourse.bass_utils import run_bass_kernel_spmd

F32 = mybir.dt.float32
BF16 = mybir.dt.bfloat16
I16 = mybir.dt.int16
OP = mybir.AluOpType

NCORES = 8
B = 262144
BS = B // NCORES          # 32768 points per core
P = 128
J = BS // P               # 256 point-columns (jout) per partition
NU = J // 16              # 16 jout-units per partition
N = 128
R = 16
TE = 2 * N * N
ES = 128                  # bf16 elems per entry (64 payload + 64 pad)
# G-side grouping zones per 16-jout unit: (group_size, group_count), in row
# order. sum(size*count) must be 16. Groups of size s share one gathered
# entry among s points (same G cell). Chosen adaptively from the input.
ZONE_CONFIGS = [
    [(4, 1), (3, 1), (2, 3), (1, 3)],  # 8 G-slots/unit (2048 quads+triples, 6144 pairs)
    [(3, 2), (2, 3), (1, 4)],   # 9 G-slots/unit (needs 4096 triples, 6144 pairs)
    [(2, 5), (1, 6)],           # 11 G-slots/unit (needs 10240 pairs)
    [(2, 4), (1, 8)],           # 12
    [(1, 16)],                  # 16 (unpaired fallback)
]
ZONES = ZONE_CONFIGS[0]
GS = sum(cnt for _, cnt in ZONES)
# chunk sizes in 16-jout units (small ends for start/tail latency)
CHUNK_U = [1, 1, 2, 2, 2, 2, 2, 2, 1, 1]
assert sum(CHUNK_U) == NU
NCH = len(CHUNK_U)
LGC = GS * P // 16        # G-list cols per unit
LHC = 16 * P // 16        # 128 H-list cols per unit
LROWS = 32


def _set_zones(zones):
    global ZONES, GS, LGC
    ZONES = zones
    assert sum(g * c for g, c in zones) == 16
    GS = sum(cnt for _, cnt in zones)
    LGC = GS * P // 16


_CACHED = None
_CACHED_ZONES = None


def _build_nc():
    nc = bacc.Bacc("TRN2")

    tbl = nc.dram_tensor("tbl", [TE, ES], BF16, kind="ExternalInput")
    lstg = nc.dram_tensor("lstg", [LROWS, NU * LGC], I16, kind="ExternalInput")
    lsth = nc.dram_tensor("lsth", [LROWS, NU * LHC], I16, kind="ExternalInput")
    w4g = nc.dram_tensor("w4g", [P, NU * 16 * 4], BF16, kind="ExternalInput")
    w4h = nc.dram_tensor("w4h", [P, NU * 16 * 4], BF16, kind="ExternalInput")
    y_pm = nc.dram_tensor("y_pm", [P, J], F32, kind="ExternalOutput")

    with tile.TileContext(nc) as tc:
        with (
            tc.tile_pool(name="per", bufs=1) as pe,
            tc.tile_pool(name="gbuf", bufs=3) as gb,
            tc.tile_pool(name="cbuf", bufs=2) as cb,
        ):
            nc.gpsimd.load_library(library_config.mlp)

            LG = pe.tile([LROWS, NU * LGC], I16)
            LH = pe.tile([LROWS, NU * LHC], I16)
            WG = pe.tile([P, NU, 16, 4], BF16)
            WH = pe.tile([P, NU, 16, 4], BF16)
            ysb = pe.tile([P, J], F32)
            # first chunk's G-list loads first so gather 0 starts early
            nc.sync.dma_start(LG[:, 0:LGC], lstg[:, 0:LGC])
            nc.sync.dma_start(LG[:, LGC:], lstg[:, LGC:])
            nc.sync.dma_start(LH[:], lsth[:])

            u0 = 0
            for ch, cu in enumerate(CHUNK_U):
                ngi = cu * GS * P          # G gather slots this chunk
                nhi = cu * 16 * P
                gG = gb.tile([P, cu * GS, ES], BF16, tag="gG",
                             padded_shape=[P, 2 * GS, ES])
                nc.gpsimd.dma_gather(
                    gG[:], tbl[:], LG[:, u0 * LGC : u0 * LGC + cu * LGC],
                    ngi, ngi, ES, queue_num=0, single_packet=False,
                )
                gH = gb.tile([P, cu * 16, ES], BF16, tag="gH",
                             padded_shape=[P, 2 * 16, ES])
                nc.gpsimd.dma_gather(
                    gH[:], tbl[:], LH[:, u0 * LHC : u0 * LHC + cu * LHC],
                    nhi, nhi, ES, queue_num=0, single_packet=False,
                )
                if ch == 0:
                    nc.sync.dma_start(
                        WG[:].rearrange("p u r c -> p (u r c)"), w4g[:]
                    )
                    nc.sync.dma_start(
                        WH[:].rearrange("p u r c -> p (u r c)"), w4h[:]
                    )

                # ---- G side ----
                # DVE APs allow at most 3 free dims after adjacent-stride
                # merging; the pair views' unit stride (GS slots) cannot merge
                # with the slot dim, so loop over the chunk's units (<= 2).
                gGv = gG[:].rearrange("p (u s) e -> p u s e", s=GS)
                uG = cb.tile([P, cu, 16, R], BF16, tag="uG",
                             padded_shape=[P, 2, 16, R])
                for u in range(cu):
                    sbase = 0
                    rbase = 0
                    for zi, (gsz, cnt) in enumerate(ZONES):
                        # cnt groups of gsz points sharing one gathered slot:
                        # jout rows rbase..rbase+gsz*cnt (i-major, sub-minor)
                        gp = (
                            gGv[:, u, sbase : sbase + cnt, 0:64]
                            .unsqueeze(2)
                            .broadcast_to([P, cnt, gsz, 64])
                            .rearrange("p i s (k c) -> p i s k c", c=4)
                        )
                        wp = (
                            WG[:, u0 + u, rbase : rbase + gsz * cnt]
                            .rearrange("p (i s) c -> p i s c", s=gsz)
                            .unsqueeze(3)
                            .broadcast_to([P, cnt, gsz, R, 4])
                        )
                        mp = cb.tile([P, cnt, gsz, R, 4], BF16,
                                     tag=f"mp{u}z{zi}")
                        nc.vector.tensor_tensor(mp[:], gp, wp, OP.mult)
                        m2p = cb.tile([P, cnt, gsz, R, 2], BF16,
                                      tag=f"m2p{u}z{zi}")
                        nc.vector.tensor_tensor(
                            m2p[:], mp[:, :, :, :, 0:2], mp[:, :, :, :, 2:4],
                            OP.add,
                        )
                        nc.vector.tensor_tensor(
                            uG[:, u, rbase : rbase + gsz * cnt].rearrange(
                                "p (i s) k -> p i s k", s=gsz
                            ),
                            m2p[:, :, :, :, 0],
                            m2p[:, :, :, :, 1],
                            OP.add,
                        )
                        sbase += cnt
                        rbase += gsz * cnt
                # ---- H side (one slot per jout) ----
                gh = (
                    gH[:]
                    .rearrange("p (u r) e -> p u r e", r=16)[:, :, :, 0:64]
                    .rearrange("p u r (k c) -> p u r k c", c=4)
                )
                wh = (
                    WH[:, u0 : u0 + cu]
                    .unsqueeze(3)
                    .broadcast_to([P, cu, 16, R, 4])
                )
                mh = cb.tile([P, cu, 16, R, 4], BF16, tag="mh",
                             padded_shape=[P, 2, 16, R, 4])
                nc.vector.tensor_tensor(mh[:], gh, wh, OP.mult)
                m2h = cb.tile([P, cu, 16, R, 2], BF16, tag="m2h",
                              padded_shape=[P, 2, 16, R, 2])
                nc.vector.tensor_tensor(
                    m2h[:], mh[:, :, :, :, 0:2], mh[:, :, :, :, 2:4], OP.add
                )
                uH = cb.tile([P, cu, 16, R], BF16, tag="uH",
                             padded_shape=[P, 2, 16, R])
                nc.vector.tensor_tensor(
                    uH[:], m2h[:, :, :, :, 0], m2h[:, :, :, :, 1], OP.add
                )
                # ---- dot ----
                pr = cb.tile([P, cu, 16, R], BF16, tag="pr",
                             padded_shape=[P, 2, 16, R])
                nc.vector.tensor_tensor(pr[:], uG[:], uH[:], OP.mult)
                nc.vector.tensor_reduce(
                    ysb[:, 16 * u0 : 16 * (u0 + cu)].rearrange(
                        "p (u r) -> p u r", r=16
                    ),
                    pr[:],
                    mybir.AxisListType.X,
                    OP.add,
                )
                u0 += cu
                if ch == NCH - 2:
                    nc.sync.dma_start(
                        y_pm[:, 0 : 16 * u0], ysb[:, 0 : 16 * u0]
                    )

            nc.sync.dma_start(y_pm[:, 16 * (NU - CHUNK_U[-1]) :],
                              ysb[:, 16 * (NU - CHUNK_U[-1]) :])

    nc.finalize()
    return nc


def _build_tables(core0, core1, core2, core3):
    G = np.einsum("nc,cmk->nmk", core0[0], core1)
    H = np.einsum("cn,kmc->nmk", core3[:, :, 0], core2)
    out = np.zeros((2, N * N, ES), dtype=ml_dtypes.bfloat16)
    for t, A in enumerate((G, H)):
        Ap = np.pad(A, ((0, 1), (0, 1), (0, 0)), mode="edge")
        corn = np.stack(
            [
                Ap[0:N, 0:N],
                Ap[0:N, 1 : N + 1],
                Ap[1 : N + 1, 0:N],
                Ap[1 : N + 1, 1 : N + 1],
            ],
            axis=-1,
        )
        out[t, :, 0:64] = corn.reshape(N * N, 64).astype(ml_dtypes.bfloat16)
    return np.ascontiguousarray(out.reshape(TE, ES))


def _wrap_list(flat):
    """slot i -> [i%16, i//16], replicated to LROWS rows."""
    lw = flat.reshape(-1, 16).T
    return np.ascontiguousarray(np.tile(lw, (LROWS // 16, 1)))


def _group_by_cell(idxG):
    """Quota-aware same-cell grouping into the ZONES quotas.

    Exact-fit cells claim their own group size first (maximum flexibility),
    then larger cells decompose by largest-still-open quota, then deferred
    cells (exact-fit surplus) do the same. Returns {gsz: array
    [n_groups, gsz]} or None if any quota cannot be filled."""
    need = {g: c * P * NU for g, c in ZONES}
    gsizes = sorted((g for g in need if g > 1), reverse=True)
    order = np.argsort(idxG, kind="stable")
    sidx = idxG[order]
    bnd = np.flatnonzero(np.r_[True, sidx[1:] != sidx[:-1]])
    counts = np.diff(np.r_[bnd, len(sidx)])
    got = {g: [] for g in gsizes}
    rem = {g: need[g] for g in gsizes}
    singles = []

    def take(members):
        o, n = 0, len(members)
        while n - o >= 2:
            for g in gsizes:
                if rem[g] and n - o >= g:
                    got[g].append(members[o : o + g])
                    rem[g] -= 1
                    o += g
                    break
            else:
                break
        for i in range(o, n):
            singles.append(members[i])

    buckets = {}
    for s, c in zip(bnd, counts):
        buckets.setdefault(c, []).append(order[s : s + c])
    deferred = buckets.pop(1, [])
    for g in gsizes:                        # exact fits first
        cells = buckets.pop(g, [])
        k = min(len(cells), rem[g])
        got[g].extend(cells[:k])
        rem[g] -= k
        deferred.extend(cells[k:])
    for c in sorted(buckets, reverse=True):  # big cells, descending
        for mem in buckets[c]:
            take(mem)
    for mem in deferred:
        take(mem)
    if any(rem.values()):
        return None
    out = {
        g: np.stack(got[g]) if got[g] else np.empty((0, g), dtype=np.int64)
        for g in gsizes
    }
    singles = np.array(singles, dtype=np.int64)
    assert len(singles) == need.get(1, 0)
    out[1] = singles.reshape(-1, 1)
    return out


def _prep_core(idxG, idxH, wG, wH):
    """Group points on the G cell per ZONES and assign to
    (partition, unit, row). Returns lstg, lsth, w4g, w4h, perm where
    perm[p, jout] = original point index at that output position."""
    groups = _group_by_cell(idxG)
    assert groups is not None

    # assignment: unit-major, then partition; per unit the jout rows are
    # zone-ordered (each zone's groups i-major, sub-minor)
    perm_parts = []        # [p, u, rows] pieces per zone
    gcell_parts = []       # [u, slots, p] pieces per zone
    for gsz, cnt in ZONES:
        grp = groups[gsz].reshape(NU, P, cnt, gsz)   # [u, p, i, s]
        perm_parts.append(
            grp.transpose(1, 0, 2, 3).reshape(P, NU, gsz * cnt)
        )
        gcell_parts.append(
            idxG[grp[:, :, :, 0]].transpose(0, 2, 1)  # [u, i, p]
        )
    perm = np.concatenate(perm_parts, axis=2).reshape(P, J)
    gcell = np.concatenate(gcell_parts, axis=1).astype(np.int16)  # [u, GS, p]
    lstg = _wrap_list(gcell.reshape(-1))

    # H slot list: slot i = (u * 16 + r) * P + p, cell of point perm[p, 16u+r]
    hp = idxH[perm]                                 # [p, jout]
    hcell = hp.reshape(P, NU, 16).transpose(1, 2, 0)  # [u, r, p]
    lsth = _wrap_list(np.ascontiguousarray(hcell).reshape(-1))

    # weights: w4g[p, u, r, c] = G-quad of the point at (p, u, r)
    w4g = wG[perm].reshape(P, NU, 16, 4).astype(ml_dtypes.bfloat16)
    w4h = wH[perm].reshape(P, NU, 16, 4).astype(ml_dtypes.bfloat16)
    return (
        lstg,
        lsth,
        np.ascontiguousarray(w4g.reshape(P, NU * 16 * 4)),
        np.ascontiguousarray(w4h.reshape(P, NU * 16 * 4)),
        perm,
    )


_PERMS = None


def _prep_inputs(x, core0, core1, core2, core3):
    global _PERMS
    core0 = np.asarray(core0, dtype=np.float32)
    core1 = np.asarray(core1, dtype=np.float32)
    core2 = np.asarray(core2, dtype=np.float32)
    core3 = np.asarray(core3, dtype=np.float32)
    tbl = _build_tables(core0, core1, core2, core3)

    x = np.asarray(x, dtype=np.float32)
    xc = np.clip(
        (x + np.float32(1.0)) * np.float32(0.5) * np.float32(N - 1),
        np.float32(0.0),
        np.float32(N - 1),
    )
    lo = np.minimum(np.floor(xc), np.float32(N - 2)).astype(np.int32)
    fr = xc - lo.astype(np.float32)
    a = np.float32(1.0) - fr

    idxG = (lo[:, 0] * N + lo[:, 1]).astype(np.int32)
    idxH = (N * N + lo[:, 3] * N + lo[:, 2]).astype(np.int32)
    wG = np.stack(
        [a[:, 0] * a[:, 1], a[:, 0] * fr[:, 1], fr[:, 0] * a[:, 1], fr[:, 0] * fr[:, 1]],
        axis=-1,
    )
    wH = np.stack(
        [a[:, 3] * a[:, 2], a[:, 3] * fr[:, 2], fr[:, 3] * a[:, 2], fr[:, 3] * fr[:, 2]],
        axis=-1,
    )

    # choose the most aggressive zone config the input distribution supports
    for cfg in ZONE_CONFIGS:
        _set_zones(cfg)
        if all(
            _group_by_cell(idxG[c * BS : (c + 1) * BS].astype(np.int16))
            is not None
            for c in range(NCORES)
        ):
            break

    in_maps = []
    _PERMS = []
    for cix in range(NCORES):
        s = slice(cix * BS, (cix + 1) * BS)
        lstg, lsth, w4gc, w4hc, perm = _prep_core(
            idxG[s].astype(np.int16), idxH[s].astype(np.int16), wG[s], wH[s]
        )
        _PERMS.append(perm)
        in_maps.append(
            {"tbl": tbl, "lstg": lstg, "lsth": lsth, "w4g": w4gc, "w4h": w4hc}
        )
    return in_maps


def kernel(x, core0, core1, core2, core3):
    global _CACHED, _CACHED_ZONES
    in_maps = _prep_inputs(x, core0, core1, core2, core3)  # sets ZONES
    if _CACHED is None or _CACHED_ZONES != ZONES:
        _CACHED = _build_nc()
        _CACHED_ZONES = ZONES
    nc = _CACHED
    res = run_bass_kernel_spmd(nc, in_maps, core_ids=list(range(NCORES)))
    out = np.empty(B, dtype=np.float32)
    for cix in range(NCORES):
        y_pm = np.asarray(res.results[cix]["y_pm"])   # [p, jout]
        dst = out[cix * BS : (cix + 1) * BS]
        dst[_PERMS[cix].reshape(-1)] = y_pm.reshape(-1)
    return out


# revision 17
# speedup vs baseline: 1.0554x; 1.0062x over previous
"""TT interpolation kernel with same-cell gather grouping.

y[b] = sum_k u_k * v_k with u = bilinear interp of the joint table
G(x0,x1), v of H(x3,x2); both tables are host-built from the TT cores and
shipped stacked as 256B bf16 entries [16 k x 4 corners]. The device does
one dma_gather stream plus a bf16 DVE combine (weight multiply with the
corner axis packed innermost for the 2x mode, corner add tree, u*v,
k-reduce).

The gather descriptor count is the bottleneck (exclusive DMA device,
22.76 ns per 256B descriptor / 16 engines), so points sharing a G-table
cell are GROUPED host-side: one gathered entry serves all group members,
each with its own bilinear weight quad. Per 16 output points the G side
gathers 9 slots (2 triples + 3 pairs + 4 singles, zone config chosen
adaptively from the input distribution) instead of 16; the H side keeps
one slot per point ordered by output position so every device-side read
stays affine. Host assigns points to (partition, unit, row) slots and
un-permutes y afterward. Descriptors per 32 points: 64 -> 50 (-22%).
"""

import numpy as np
import ml_dtypes

import concourse.bacc as bacc
import concourse.mybir as mybir
import concourse.tile as tile
from concourse import library_config
from concourse.bass_utils import run_bass_kernel_spmd

F32 = mybir.dt.float32
BF16 = mybir.dt.bfloat16
I16 = mybir.dt.int16
OP = mybir.AluOpType

NCORES = 8
B = 262144
BS = B // NCORES          # 32768 points per core
P = 128
J = BS // P               # 256 point-columns (jout) per partition
UROWS = 32                # jout rows per unit
NU = J // UROWS           # jout-units per partition
N = 128
R = 16
TE = 2 * N * N
ES = 128                  # bf16 elems per entry (64 payload + 64 pad)
# G-side grouping zones per 16-jout unit: (group_size, group_count), in row
# order. sum(size*count) must be 16. Groups of size s share one gathered
# entry among s points (same G cell). Chosen adaptively from the input.
ZONE_CONFIGS = [
    [(4, 2), (3, 3), (2, 5), (1, 5)],  # 15 G-slots/32 (2048 quads, 3072 triples)
    [(4, 2), (3, 2), (2, 6), (1, 6)],  # 16 (the previously shipped mix)
    [(3, 4), (2, 6), (1, 8)],          # 18
    [(2, 10), (1, 12)],                # 22
    [(1, 32)],                         # 32 (unpaired fallback)
]
ZONES = ZONE_CONFIGS[0]
GS = sum(cnt for _, cnt in ZONES)
# chunk sizes in 16-jout units (small ends for start/tail latency)
CHUNK_U = [1, 1, 2, 2, 1, 1]
assert sum(CHUNK_U) == NU
NCH = len(CHUNK_U)
LGC = GS * P // 16        # G-list cols per unit
LHC = UROWS * P // 16     # H-list cols per unit
LROWS = 32


def _set_zones(zones):
    global ZONES, GS, LGC
    ZONES = zones
    assert sum(g * c for g, c in zones) == UROWS
    GS = sum(cnt for _, cnt in zones)
    LGC = GS * P // 16


_CACHED = None
_CACHED_ZONES = None


def _build_nc():
    nc = bacc.Bacc("TRN2")

    tbl = nc.dram_tensor("tbl", [TE, ES], BF16, kind="ExternalInput")
    lstg = nc.dram_tensor("lstg", [LROWS, NU * LGC], I16, kind="ExternalInput")
    lsth = nc.dram_tensor("lsth", [LROWS, NU * LHC], I16, kind="ExternalInput")
    w4g = nc.dram_tensor("w4g", [P, NU * UROWS * 4], BF16, kind="ExternalInput")
    w4h = nc.dram_tensor("w4h", [P, NU * UROWS * 4], BF16, kind="ExternalInput")
    y_pm = nc.dram_tensor("y_pm", [P, J], F32, kind="ExternalOutput")

    with tile.TileContext(nc) as tc:
        with (
            tc.tile_pool(name="per", bufs=1) as pe,
            tc.tile_pool(name="gbuf", bufs=3) as gb,
            tc.tile_pool(name="cbuf", bufs=2) as cb,
        ):
            nc.gpsimd.load_library(library_config.mlp)

            LG = pe.tile([LROWS, NU * LGC], I16)
            LH = pe.tile([LROWS, NU * LHC], I16)
            WG = pe.tile([P, NU, UROWS, 4], BF16)
            WH = pe.tile([P, NU, UROWS, 4], BF16)
            ysb = pe.tile([P, J], F32)
            # first chunk's G-list loads first so gather 0 starts early
            nc.sync.dma_start(LG[:, 0:LGC], lstg[:, 0:LGC])
            nc.sync.dma_start(LG[:, LGC:], lstg[:, LGC:])
            nc.sync.dma_start(LH[:], lsth[:])

            u0 = 0
            for ch, cu in enumerate(CHUNK_U):
                ngi = cu * GS * P          # G gather slots this chunk
                nhi = cu * UROWS * P
                gG = gb.tile([P, cu * GS, ES], BF16, tag="gG",
                             padded_shape=[P, 2 * GS, ES])
                nc.gpsimd.dma_gather(
                    gG[:], tbl[:], LG[:, u0 * LGC : u0 * LGC + cu * LGC],
                    ngi, ngi, ES, queue_num=0, single_packet=False,
                )
                hparts = (
                    [(0, 16), (16, 32)]
                    if ch == NCH - 1
                    else [(r, r + UROWS) for r in range(0, cu * UROWS, UROWS)]
                )
                ghs = []
                for r0, r1 in hparts:
                    nh = (r1 - r0) * P
                    gHh = gb.tile([P, r1 - r0, ES], BF16, tag=f"gH{r0}",
                                  padded_shape=[P, UROWS, ES])
                    nc.gpsimd.dma_gather(
                        gHh[:], tbl[:],
                        LH[:, u0 * LHC + 8 * r0 : u0 * LHC + 8 * r1],
                        nh, nh, ES, queue_num=0, single_packet=False,
                    )
                    ghs.append((gHh, r0, r1))
                if ch == 0:
                    nc.sync.dma_start(
                        WG[:].rearrange("p u r c -> p (u r c)"), w4g[:]
                    )
                    nc.sync.dma_start(
                        WH[:].rearrange("p u r c -> p (u r c)"), w4h[:]
                    )

                # ---- G side ----
                # DVE APs allow at most 3 free dims after adjacent-stride
                # merging; the pair views' unit stride (GS slots) cannot merge
                # with the slot dim, so loop over the chunk's units (<= 2).
                gGv = gG[:].rearrange("p (u s) e -> p u s e", s=GS)
                uG = cb.tile([P, cu, UROWS, R], BF16, tag="uG",
                             padded_shape=[P, 2, UROWS, R])
                for u in range(cu):
                    sbase = 0
                    rbase = 0
                    for zi, (gsz, cnt) in enumerate(ZONES):
                        # cnt groups of gsz points sharing one gathered slot:
                        # jout rows rbase..rbase+gsz*cnt (i-major, sub-minor)
                        gp = (
                            gGv[:, u, sbase : sbase + cnt, 0:64]
                            .unsqueeze(2)
                            .broadcast_to([P, cnt, gsz, 64])
                            .rearrange("p i s (k c) -> p i s k c", c=4)
                        )
                        wp = (
                            WG[:, u0 + u, rbase : rbase + gsz * cnt]
                            .rearrange("p (i s) c -> p i s c", s=gsz)
                            .unsqueeze(3)
                            .broadcast_to([P, cnt, gsz, R, 4])
                        )
                        mp = cb.tile([P, cnt, gsz, R, 4], BF16,
                                     tag=f"mp{u}z{zi}")
                        nc.vector.tensor_tensor(mp[:], gp, wp, OP.mult)
                        m2p = cb.tile([P, cnt, gsz, R, 2], BF16,
                                      tag=f"m2p{u}z{zi}")
                        nc.vector.tensor_tensor(
                            m2p[:], mp[:, :, :, :, 0:2], mp[:, :, :, :, 2:4],
                            OP.add,
                        )
                        nc.vector.tensor_tensor(
                            uG[:, u, rbase : rbase + gsz * cnt].rearrange(
                                "p (i s) k -> p i s k", s=gsz
                            ),
                            m2p[:, :, :, :, 0],
                            m2p[:, :, :, :, 1],
                            OP.add,
                        )
                        sbase += cnt
                        rbase += gsz * cnt
                # ---- H side (one slot per jout) + dot, per H part ----
                uGf = uG[:].rearrange("p u r k -> p (u r) k")
                whf = WH[:, u0 : u0 + cu].rearrange("p u r c -> p (u r) c")
                for gHh, r0, r1 in ghs:
                    nr = r1 - r0
                    gh = gHh[:, :, 0:64].rearrange("p r (k c) -> p r k c", c=4)
                    wh = whf[:, r0:r1].unsqueeze(2).broadcast_to([P, nr, R, 4])
                    mh = cb.tile([P, nr, R, 4], BF16, tag=f"mh{r0}",
                                 padded_shape=[P, UROWS, R, 4])
                    nc.vector.tensor_tensor(mh[:], gh, wh, OP.mult)
                    m2h = cb.tile([P, nr, R, 2], BF16, tag=f"m2h{r0}",
                                  padded_shape=[P, UROWS, R, 2])
                    nc.vector.tensor_tensor(
                        m2h[:], mh[:, :, :, 0:2], mh[:, :, :, 2:4], OP.add
                    )
                    uH = cb.tile([P, nr, R], BF16, tag=f"uH{r0}",
                                 padded_shape=[P, UROWS, R])
                    nc.vector.tensor_tensor(
                        uH[:], m2h[:, :, :, 0], m2h[:, :, :, 1], OP.add
                    )
                    pr = cb.tile([P, nr, R], BF16, tag=f"pr{r0}",
                                 padded_shape=[P, UROWS, R])
                    nc.vector.tensor_tensor(
                        pr[:], uGf[:, UROWS * (u0 - u0) + r0 : r1], uH[:], OP.mult
                    )
                    nc.vector.tensor_reduce(
                        ysb[:, UROWS * u0 + r0 : UROWS * u0 + r1],
                        pr[:],
                        mybir.AxisListType.X,
                        OP.add,
                    )
                u0 += cu
                if ch == NCH - 2:
                    nc.sync.dma_start(
                        y_pm[:, 0 : UROWS * u0], ysb[:, 0 : UROWS * u0]
                    )

            nc.sync.dma_start(y_pm[:, 16 * (NU - CHUNK_U[-1]) :],
                              ysb[:, 16 * (NU - CHUNK_U[-1]) :])

    nc.finalize()
    return nc


def _build_tables(core0, core1, core2, core3):
    G = np.einsum("nc,cmk->nmk", core0[0], core1)
    H = np.einsum("cn,kmc->nmk", core3[:, :, 0], core2)
    out = np.zeros((2, N * N, ES), dtype=ml_dtypes.bfloat16)
    for t, A in enumerate((G, H)):
        Ap = np.pad(A, ((0, 1), (0, 1), (0, 0)), mode="edge")
        corn = np.stack(
            [
                Ap[0:N, 0:N],
                Ap[0:N, 1 : N + 1],
                Ap[1 : N + 1, 0:N],
                Ap[1 : N + 1, 1 : N + 1],
            ],
            axis=-1,
        )
        out[t, :, 0:64] = corn.reshape(N * N, 64).astype(ml_dtypes.bfloat16)
    return np.ascontiguousarray(out.reshape(TE, ES))


def _wrap_list(flat):
    """slot i -> [i%16, i//16], replicated to LROWS rows."""
    lw = flat.reshape(-1, 16).T
    return np.ascontiguousarray(np.tile(lw, (LROWS // 16, 1)))


def _group_by_cell(idxG):
    """Quota-aware same-cell grouping into the ZONES quotas.

    Exact-fit cells claim their own group size first (maximum flexibility),
    then larger cells decompose by largest-still-open quota, then deferred
    cells (exact-fit surplus) do the same. Returns {gsz: array
    [n_groups, gsz]} or None if any quota cannot be filled."""
    need = {g: c * P * NU for g, c in ZONES}
    gsizes = sorted((g for g in need if g > 1), reverse=True)
    order = np.argsort(idxG, kind="stable")
    sidx = idxG[order]
    bnd = np.flatnonzero(np.r_[True, sidx[1:] != sidx[:-1]])
    counts = np.diff(np.r_[bnd, len(sidx)])
    got = {g: [] for g in gsizes}
    rem = {g: need[g] for g in gsizes}
    singles = []

    def take(members):
        o, n = 0, len(members)
        while n - o >= 2:
            for g in gsizes:
                if rem[g] and n - o >= g:
                    got[g].append(members[o : o + g])
                    rem[g] -= 1
                    o += g
                    break
            else:
                break
        for i in range(o, n):
            singles.append(members[i])

    buckets = {}
    for s, c in zip(bnd, counts):
        buckets.setdefault(c, []).append(order[s : s + c])
    deferred = buckets.pop(1, [])
    for g in gsizes:                        # exact fits first
        cells = buckets.pop(g, [])
        k = min(len(cells), rem[g])
        got[g].extend(cells[:k])
        rem[g] -= k
        deferred.extend(cells[k:])
    for c in sorted(buckets, reverse=True):  # big cells, descending
        for mem in buckets[c]:
            take(mem)
    for mem in deferred:
        take(mem)
    if any(rem.values()):
        return None
    out = {
        g: np.stack(got[g]) if got[g] else np.empty((0, g), dtype=np.int64)
        for g in gsizes
    }
    singles = np.array(singles, dtype=np.int64)
    assert len(singles) == need.get(1, 0)
    out[1] = singles.reshape(-1, 1)
    return out


def _prep_core(idxG, idxH, wG, wH):
    """Group points on the G cell per ZONES and assign to
    (partition, unit, row). Returns lstg, lsth, w4g, w4h, perm where
    perm[p, jout] = original point index at that output position."""
    groups = _group_by_cell(idxG)
    assert groups is not None

    # assignment: unit-major, then partition; per unit the jout rows are
    # zone-ordered (each zone's groups i-major, sub-minor)
    perm_parts = []        # [p, u, rows] pieces per zone
    gcell_parts = []       # [u, slots, p] pieces per zone
    for gsz, cnt in ZONES:
        grp = groups[gsz].reshape(NU, P, cnt, gsz)   # [u, p, i, s]
        perm_parts.append(
            grp.transpose(1, 0, 2, 3).reshape(P, NU, gsz * cnt)
        )
        gcell_parts.append(
            idxG[grp[:, :, :, 0]].transpose(0, 2, 1)  # [u, i, p]
        )
    perm = np.concatenate(perm_parts, axis=2).reshape(P, J)
    gcell = np.concatenate(gcell_parts, axis=1).astype(np.int16)  # [u, GS, p]
    lstg = _wrap_list(gcell.reshape(-1))

    # H slot list: slot i = (u * 16 + r) * P + p, cell of point perm[p, 16u+r]
    hp = idxH[perm]                                 # [p, jout]
    hcell = hp.reshape(P, NU, UROWS).transpose(1, 2, 0)  # [u, r, p]
    lsth = _wrap_list(np.ascontiguousarray(hcell).reshape(-1))

    # weights: w4g[p, u, r, c] = G-quad of the point at (p, u, r)
    w4g = wG[perm].reshape(P, NU, UROWS, 4).astype(ml_dtypes.bfloat16)
    w4h = wH[perm].reshape(P, NU, UROWS, 4).astype(ml_dtypes.bfloat16)
    return (
        lstg,
        lsth,
        np.ascontiguousarray(w4g.reshape(P, NU * UROWS * 4)),
        np.ascontiguousarray(w4h.reshape(P, NU * UROWS * 4)),
        perm,
    )


_PERMS = None


def _prep_inputs(x, core0, core1, core2, core3):
    global _PERMS
    core0 = np.asarray(core0, dtype=np.float32)
    core1 = np.asarray(core1, dtype=np.float32)
    core2 = np.asarray(core2, dtype=np.float32)
    core3 = np.asarray(core3, dtype=np.float32)
    tbl = _build_tables(core0, core1, core2, core3)

    x = np.asarray(x, dtype=np.float32)
    xc = np.clip(
        (x + np.float32(1.0)) * np.float32(0.5) * np.float32(N - 1),
        np.float32(0.0),
        np.float32(N - 1),
    )
    lo = np.minimum(np.floor(xc), np.float32(N - 2)).astype(np.int32)
    fr = xc - lo.astype(np.float32)
    a = np.float32(1.0) - fr

    idxG = (lo[:, 0] * N + lo[:, 1]).astype(np.int32)
    idxH = (N * N + lo[:, 3] * N + lo[:, 2]).astype(np.int32)
    wG = np.stack(
        [a[:, 0] * a[:, 1], a[:, 0] * fr[:, 1], fr[:, 0] * a[:, 1], fr[:, 0] * fr[:, 1]],
        axis=-1,
    )
    wH = np.stack(
        [a[:, 3] * a[:, 2], a[:, 3] * fr[:, 2], fr[:, 3] * a[:, 2], fr[:, 3] * fr[:, 2]],
        axis=-1,
    )

    # choose the most aggressive zone config the input distribution supports
    for cfg in ZONE_CONFIGS:
        _set_zones(cfg)
        if all(
            _group_by_cell(idxG[c * BS : (c + 1) * BS].astype(np.int16))
            is not None
            for c in range(NCORES)
        ):
            break

    in_maps = []
    _PERMS = []
    for cix in range(NCORES):
        s = slice(cix * BS, (cix + 1) * BS)
        lstg, lsth, w4gc, w4hc, perm = _prep_core(
            idxG[s].astype(np.int16), idxH[s].astype(np.int16), wG[s], wH[s]
        )
        _PERMS.append(perm)
        in_maps.append(
            {"tbl": tbl, "lstg": lstg, "lsth": lsth, "w4g": w4gc, "w4h": w4hc}
        )
    return in_maps


def kernel(x, core0, core1, core2, core3):
    global _CACHED, _CACHED_ZONES
    in_maps = _prep_inputs(x, core0, core1, core2, core3)  # sets ZONES
    if _CACHED is None or _CACHED_ZONES != ZONES:
        _CACHED = _build_nc()
        _CACHED_ZONES = ZONES
    nc = _CACHED
    res = run_bass_kernel_spmd(nc, in_maps, core_ids=list(range(NCORES)))
    out = np.empty(B, dtype=np.float32)
    for cix in range(NCORES):
        y_pm = np.asarray(res.results[cix]["y_pm"])   # [p, jout]
        dst = out[cix * BS : (cix + 1) * BS]
        dst[_PERMS[cix].reshape(-1)] = y_pm.reshape(-1)
    return out
